# revision 38
# baseline (speedup 1.0000x reference)
"""AttentionGate kernel for Trainium2 (8 NeuronCores, pure data parallel).

Reference computation (per pixel p, channels c):
    t[p] = sum_c input_[p,c]*wt[c] + bt
    g[p] = sum_c gating [p,c]*wg[c] + bg
    x[p] = sigmoid(w2 * relu(t[p]+g[p]) + b2)
    out[p,c] = input_[p,c] * x[p]

HBM-bandwidth-bound kernel, 40 MB of HBM I/O per core (16 bf16 input_ +
8 fp8 gating + 16 bf16 out; ~24.5 GB/s per SDMA engine x16 => ~103us of
DMA engine time).  Host stores both inputs CHANNEL-MAJOR so the
per-pixel 512-channel dot runs on the otherwise-idle TensorEngine:
  - input_ as bf16 chunks [2*128ch, 32768px]: 2 accumulating matmuls
    (lhsT = wt-chunk [128,1], rhs [128,512]) per 512-px slice; the same
    tiles feed the final gate multiply.
  - gating as fp8-e4m3 [128part, 2, 32768] (two 128-ch planes per
    partition, plane stride PX): ONE DoubleRow matmul per slice
    accumulates both chunk dots (dual-fp8 LDWEIGHTS needs the weight
    planes 16B apart, hence the padded [128,2,16] weight tile).  fp8 g
    only perturbs the sigmoid argument (w2*g_err ~ 1e-3); measured
    end-to-end rel-err 1.383e-2 < 2e-2 gate (bit-exact vs the numpy
    model of this pipeline).
The relu FOLDS AWAY: sigmoid(w2*relu(z)+b2) == max/min(sigmoid(w2*z +
w2*(bt+bg)+b2), sigmoid(b2)) by monotonicity, so ACT runs a single
sigmoid on the [1,N] psum dot row -> bf16; the PE replicates that row
to 128 partitions via an outer-product matmul (lhsT=ones[1,128]); DVE
applies the clamp and the gate multiply in one scalar_tensor_tensor
(rep max sigmoid(b2)) * x per output chunk.  Out-DMAs are emitted one
block late on the ACT ring so their wait on the DVE muls never
head-of-line blocks the next sigmoid; the last blocks' stores issue
immediately from the then-idle SP ring.  A few dead matmuls at t=0 flip
the PE's HAM clock gate to 2.4GHz before real dots arrive.  Host
pre/post-transposes layouts (pure data movement, no math).

512-px sub-blocks with 1-bank PSUM tiles x4 bufs (vs 1024-px x2) halve
every cross-engine wait and double the PE pipeline depth -- worth ~10us
over the 1024-px variant.  Engine busy per core (measured): DMA
~103-109us (bottleneck), PE ~98us (HAM K=4/8 cycling; ~55us fully
warm), DVE ~82us, ACT ~54us; exec ~123-131us vs 190us for the
pixel-major DVE-dot baseline.

Sharding: batch dim 16 -> 2 batches per core, weights replicated.
"""

import sys

import numpy as np

for _p in ("/opt/trn_rl_repo", "/opt/trn_rl_repo/concourse"):
    if _p not in sys.path:
        sys.path.append(_p)

B, H, W, C = 16, 128, 128, 256
NCORES = 8
PX = (B // NCORES) * H * W             # pixels per core = 32768
P = 128                                # partitions
XCH = 2                                # bf16 input_ chunks of 128 channels
OCH = 2                                # output channel chunks
SB = 512                               # pixels per compute sub-block
MMN = 512                              # matmul free-dim slice (psum bank cap)
# DMA block sizes (pixels): short head so compute starts early, short
# tail so the final store waits on a small chain.
BLOCKS = [512, 1024, 2048] + [4096] * 7 + [512]
assert sum(BLOCKS) == PX
# out-DMAs of the last blocks issue immediately from the then-idle SP
# ring instead of the deferred ACT-ring path
TAIL_SYNC_BLOCKS = 2
# dummy matmuls issued while the first input DMA is in flight: ~4us of
# sustained PE activity flips the HAM clock gate to K=8/8 (2.4 GHz)
# before the first real dot, instead of paying ~2x on the whole head
WARMUP_MMS = 20

_PATCHED = False


def _apply_compat_patches():
    """Work around two ISA-encoding gaps in this container's neuronxcc walrus:

    1. EVENT_SEMAPHORE_RANGE_CLEAR (emitted by the TileContext teardown's
       sem_clear) fails codegen with "ISA wrong length".  Re-execution is
       safe without it (verified on HW), so skip the clear.
    2. The teardown drain carries one sem-wait per logical processor; this
       walrus rejects >1 sync-wait command on a NO_STRUCT ctrl instruction
       ("Too many sync wait commands").  Split the final clock wait into one
       NOP per processor instead.
    """
    global _PATCHED
    if _PATCHED:
        return
    _PATCHED = True

    import concourse.bass as bass
    import concourse.tile as tile_mod
    from bass_rust import ScopedClock, VectorClock
    from concourse.bass import SemaphoreHandle, compact_to_ranges

    def patched_clear(self, sems):
        if not sems:
            return
        sem_nums = [s.num if isinstance(s, SemaphoreHandle) else s for s in sems]
        for sem_range in compact_to_ranges(sem_nums):
            assert self._state.free_isdisjoint(sem_range)
            self.gpsimd.dma_reset(sem_range)
        self._state.prepend_free_semaphores(sem_nums)
        for poison_set in self._tile_sem_poison_stack:
            poison_set.update(sem_nums)

    bass.Bass.clear_and_free_semaphores = patched_clear

    def patched_drain_and_barrier(self, tick_clock, wait_clock):
        gc = tick_clock.global_clock
        for p in range(len(gc)):
            if gc[p] <= 0:
                continue
            vc = VectorClock()
            vc.require_at_least(p, gc[p])
            di = self.nc.sync.nop(nofuse=True)
            wait_clock.add_sem_waits(di.ins, ScopedClock({None: vc}))
        assert self.sems is not None
        popped = self.nc._tile_sem_poison_stack.pop()
        assert popped is self._sem_poison
        # bookkeeping only: recycle sem ids; no dma_reset (the body issues
        # no SWDGE DMAs) and no second barrier -> shorter kernel tail
        sems = list(self.sems.allocated().values())
        from concourse.bass import SemaphoreHandle
        sem_nums = [s.num if isinstance(s, SemaphoreHandle) else s for s in sems]
        self.nc._state.prepend_free_semaphores(sem_nums)
        for poison_set in self.nc._tile_sem_poison_stack:
            poison_set.update(sem_nums)

    tile_mod.TileContext._drain_and_barrier = patched_drain_and_barrier


def _split_multi_waits(nc):
    """This walrus build only encodes ONE sync-wait command per TPB
    instruction.  Hoist all-but-the-last wait of any instruction onto
    freshly inserted same-engine NoOps placed directly before it."""
    import concourse.mybir as mybir

    for f in nc.m.functions:
        for bb in f.blocks:
            insts = bb.instructions  # live list
            i = 0
            while i < len(insts):
                inst = insts[i]
                si = getattr(inst, "sync_info", None)
                if si is not None and len(si.on_wait) > 1:
                    extra, last = list(si.on_wait[:-1]), si.on_wait[-1]
                    for w in extra:
                        nop = mybir.InstNoOp(
                            name=nc.get_next_instruction_name(),
                            engine=inst.engine,
                            sync_info=mybir.SyncInfo(on_wait=[w], on_update=[]),
                            bass_nofuse=True,
                        )
                        insts.insert(i, nop)
                        i += 1
                    inst.sync_info = mybir.SyncInfo(
                        on_wait=[last], on_update=list(si.on_update)
                    )
                i += 1


def _build_program(bt, bg, w2, b2):
    import concourse.bass as bass
    import concourse.mybir as mybir
    from concourse.tile import TileContext

    nc = bass.Bass()
    bf16 = mybir.dt.bfloat16
    fp8 = mybir.dt.float8e4
    f32 = mybir.dt.float32
    xh_d = nc.declare_dram_parameter("xh", [XCH * P, PX], bf16, isOutput=False)
    gh_d = nc.declare_dram_parameter("gh", [P, 2 * PX], fp8, isOutput=False)
    wx_d = nc.declare_dram_parameter("wx", [P, XCH], bf16, isOutput=False)
    # dual-fp8 LDWEIGHTS requires the two k-tile planes >= 16B apart
    wg_d = nc.declare_dram_parameter("wg8", [P, 32], fp8, isOutput=False)
    o_d = nc.declare_dram_parameter("out", [OCH * P, PX], bf16, isOutput=True)

    xh_v = xh_d[:].rearrange("(k p) n -> k p n", k=XCH)
    gh_v = gh_d[:].rearrange("p (two n) -> p two n", two=2)
    o_v = o_d[:].rearrange("(k p) n -> k p n", k=OCH)

    DBMAX = max(BLOCKS)
    DR = mybir.MatmulPerfMode.DoubleRow
    # clamp restoring the relu: x = sigmoid(w2*relu(z)+b2) equals
    # max(sigmoid(w2*z+b2), sigmoid(b2)) for w2>0 (min for w2<0), exactly
    clamp = float(1.0 / (1.0 + np.exp(-b2)))
    clamp_op = (
        mybir.AluOpType.max if w2 >= 0 else mybir.AluOpType.min
    )

    with TileContext(nc) as tc:
        with (
            tc.tile_pool(name="wp", bufs=1) as wp,
            tc.tile_pool(name="io", bufs=4) as io,
            tc.tile_pool(name="op", bufs=3) as op,
            tc.tile_pool(name="sg", bufs=4) as sg,
            tc.psum_pool(name="pp", bufs=4) as pp,
        ):
            # weight loads ride the ACT ring so the SP ring's very first
            # issue is block-0's input prefetch
            wx = wp.tile([P, XCH], bf16)         # column k = x-chunk-k weights
            nc.scalar.dma_start(wx[:], wx_d[:])
            wg8 = wp.tile([P, 2, 16], fp8)       # plane i = g-chunk-i weights
            nc.scalar.dma_start(
                wg8[:], wg_d[:].rearrange("p (two m) -> p two m", two=2)
            )
            ones = wp.tile([1, P], bf16)         # outer-product replicator
            nc.vector.memset(ones[:], 1.0)

            # HAM warmup: dead matmuls on the tiny `ones` tile -- ready
            # within ~4us of kernel start, so the burst completes during
            # the first input DMA's flight and block-0 dots start warm
            # with no queue delay
            wrep = pp.tile([P, SB], f32, tag="rep", name="wrep")
            for _ in range(WARMUP_MMS):
                nc.tensor.matmul(
                    wrep[:, 0:P], lhsT=ones[:], rhs=ones[:],
                    start=True, stop=True,
                )

            pending = []                         # deferred out-DMAs (ACT)
            pending_sync = []                    # tail out-DMAs (SP ring)
            off = 0
            for bi, sz in enumerate(BLOCKS):
                in_tail = bi >= len(BLOCKS) - TAIL_SYNC_BLOCKS
                blk = slice(off, off + sz)
                xins = []
                for k in range(XCH):
                    xk = io.tile([P, DBMAX], bf16, tag=f"xin{k}", name=f"xin{k}")
                    nc.sync.dma_start(xk[:, 0:sz], xh_v[k, :, blk])
                    xins.append(xk)
                gt = io.tile([P, 2, DBMAX], fp8, tag="gin", name="gin")
                nc.sync.dma_start(gt[:, :, 0:sz], gh_v[:, :, blk])
                # tail stores ride the SP ring, but only BEHIND the next
                # block's input prefetch so they never head-of-line block it
                for dst, src in pending_sync:
                    nc.sync.dma_start(dst, src)
                pending_sync = []
                outs = [
                    op.tile([P, DBMAX], bf16, tag=f"out{c}", name=f"out{c}")
                    for c in range(OCH)
                ]
                for s0 in range(0, sz, SB):
                    sbn = min(SB, sz - s0)
                    # per-pixel 512-dot: 2 bf16 matmuls (input_) + 1 fp8
                    # DoubleRow matmul (gating, both chunks) per 512-px col
                    dot = pp.tile([1, SB], f32, tag="dot")
                    for h in range(0, sbn, MMN):
                        cols = slice(s0 + h, s0 + h + MMN)
                        for k in range(XCH):
                            nc.tensor.matmul(
                                dot[:, h : h + MMN],
                                lhsT=wx[:, k : k + 1],
                                rhs=xins[k][:, cols],
                                start=(k == 0),
                                stop=False,
                            )
                        nc.tensor.matmul(
                            dot[:, h : h + MMN],
                            lhsT=wg8[:, :, 0:1],
                            rhs=gt[:, :, cols],
                            start=False,
                            stop=True,
                            perf_mode=DR,
                        )
                    # sigmoid(w2*relu(z)+b2) == clamp(sigmoid(w2*(z+bt+bg)+b2)
                    # against sigmoid(b2)) -- the relu folds into a single
                    # ACT sigmoid + a max/min inside the DVE gate multiply
                    xsg = sg.tile([1, SB], bf16, tag="xsg")
                    nc.scalar.activation(
                        xsg[:, 0:sbn], dot[:, 0:sbn],
                        mybir.ActivationFunctionType.Sigmoid,
                        bias=float(w2 * (bt + bg) + b2), scale=float(w2),
                    )
                    # replicate sigmoid row to 128 partitions: ones^T @ xsg
                    rep = pp.tile([P, SB], f32, tag="rep")
                    for h in range(0, sbn, MMN):
                        nc.tensor.matmul(
                            rep[:, h : h + MMN],
                            lhsT=ones[:],
                            rhs=xsg[:, h : h + MMN],
                            start=True,
                            stop=True,
                        )
                    if s0 == 0:
                        # previous block's out-DMAs, emitted here so their
                        # wait on DVE muls is already satisfied and never
                        # head-of-line blocks the ACT ring
                        for dst, src in pending:
                            nc.scalar.dma_start(dst, src)
                        pending = []
                    for c in range(OCH):
                        nc.vector.scalar_tensor_tensor(
                            out=outs[c][:, s0 : s0 + sbn],
                            in0=rep[:, 0:sbn],
                            scalar=clamp,
                            in1=xins[c][:, s0 : s0 + sbn],
                            op0=clamp_op,
                            op1=mybir.AluOpType.mult,
                        )
                for c in range(OCH):
                    if in_tail:
                        pending_sync.append((o_v[c, :, blk], outs[c][:, 0:sz]))
                    else:
                        pending.append((o_v[c, :, blk], outs[c][:, 0:sz]))
                off += sz
            for dst, src in pending_sync:
                nc.sync.dma_start(dst, src)
            for dst, src in pending:
                nc.scalar.dma_start(dst, src)
    _split_multi_waits(nc)
    return nc


def _f32_to_bf16_bits(a):
    """Round-to-nearest-even f32 -> bf16 bit pattern (uint16)."""
    u = np.ascontiguousarray(a, dtype=np.float32).view(np.uint32)
    return ((u + 0x7FFF + ((u >> 16) & 1)) >> 16).astype(np.uint16)


def _bf16_to_f32(a):
    """Exact bf16 -> f32 up-conversion via bit manipulation."""
    u = np.ascontiguousarray(a).view(np.uint16).astype(np.uint32)
    return (u << 16).view(np.float32)


def kernel(**inputs):
    _apply_compat_patches()
    import ml_dtypes
    from concourse.bass_utils import run_bass_kernel_spmd

    x = np.asarray(inputs["input_"], dtype=np.float32)
    g = np.asarray(inputs["gating_signal"], dtype=np.float32)
    wt = np.asarray(inputs["wt"], dtype=np.float32)
    wg = np.asarray(inputs["wg"], dtype=np.float32)
    bt = float(np.asarray(inputs["bt"]))
    bg = float(np.asarray(inputs["bg"]))
    w2 = float(np.asarray(inputs["w2"]))
    b2 = float(np.asarray(inputs["b2"]))

    nc = _build_program(bt, bg, w2, b2)

    # input_: channel-major bf16, rows 0-255 = x channels
    xb = _f32_to_bf16_bits(x).reshape(NCORES, PX, C)
    xh = np.empty((NCORES, C, PX), dtype=np.uint16)
    for i in range(NCORES):
        xh[i] = xb[i].T
    xh16 = xh.view(ml_dtypes.bfloat16)

    # gating: channel-major fp8e4m3 as [128 part, 2 planes, PX]
    g8 = g.astype(ml_dtypes.float8_e4m3).reshape(NCORES, PX, C)
    gh = np.empty((NCORES, P, 2, PX), dtype=ml_dtypes.float8_e4m3)
    for i in range(NCORES):
        gcm = g8[i].T                      # [256, PX]
        gh[i, :, 0, :] = gcm[0:P]
        gh[i, :, 1, :] = gcm[P : 2 * P]
    gh = gh.reshape(NCORES, P, 2 * PX)

    wx = np.ascontiguousarray(
        _f32_to_bf16_bits(wt).reshape(XCH, P).T
    ).view(ml_dtypes.bfloat16)                                   # [128, 2]
    wg8 = np.zeros((P, 32), dtype=ml_dtypes.float8_e4m3)
    wg8[:, 0] = wg[0:P].astype(ml_dtypes.float8_e4m3)            # plane 0
    wg8[:, 16] = wg[P : 2 * P].astype(ml_dtypes.float8_e4m3)     # plane 1

    in_maps = [
        {"xh": xh16[i], "gh": gh[i], "wx": wx, "wg8": wg8} for i in range(NCORES)
    ]
    res = run_bass_kernel_spmd(nc, in_maps, list(range(NCORES)))

    out = np.empty((NCORES, PX, C), dtype=np.float32)
    for i in range(NCORES):
        o_cm = np.asarray(res.results[i]["out"]).view(np.uint16)  # [256, PX]
        out[i] = _bf16_to_f32(np.ascontiguousarray(o_cm.T))
    return out.reshape(B, H, W, C)
